# revision 86
# baseline (speedup 1.0000x reference)
"""Trainium2 Bass kernel for multi-head attention with RoPE.

Problem: B=2, T=2048, D=1024, H=16 heads (hd=64), fused qkv projection,
RoPE on q/k, softmax attention, output projection.

Sharding over 8 NeuronCores: data parallel on batch (2) x tensor parallel
on heads (4 groups of 4 heads). Core c handles batch c//4, head group c%4:
 - qkv projection: w_qkv column-split per head group (q,k,v cols of its heads)
 - attention for its 4 heads
 - out projection: w_out row-split; per-core partial [T, D] outputs are
   summed on the host (the TP all-reduce is folded into the host gather).

Device-side layout (per core):
 - x pre-transposed on host: xT [D, T] (d on partitions, matmul-ready).
 - q,k computed TRANSPOSED ([c, t]) in 8 per-(cb,hb) tiles [128, 1024]
   (split tiles avoid whole-tile false dependencies in the Tile tracker).
 - RoPE: partition-swap via SBUF->SBUF DMA (gpsimd software-DGE queue) +
   sign folded into the host sin table; temps in bf16 (2x DVE add,
   half-size swap DMA). The q/k BIAS rides the rope path: rope is linear,
   so host-precomputed rope(b) tables are pre-filled into the swap target
   and the swap DMAs ACCUMULATE onto them -- no bias matmuls at all.
 - scores run as K=128 matmuls with the ZERO-PAD ON THE Q SIDE: lhsT is the
   unpadded two-head k tile, rhs is a per-unit q_pad tile whose other-head
   rows are zero (pre-zeroed parity tiles; per unit only a [64,1024] copy,
   hoisted into the previous unit).
 - softmax denominator from an all-ones column appended to v; no max
   subtraction (logits ~N(0,1)).
 - normalize: reciprocal (DVE) -> row-64 shift DMA -> gpsimd partition
   broadcast -> DVE mul.
 - DMA discipline (measured: ~670ns per issue on a sequencer, ~80ns per
   descriptor = SBUF partition row per queue, ~24GB/s per queue):
   startup loads alternate the sync/scalar HWDGE queues, all bulk tensors
   are host-packed so every DMA moves >=2KB per row, ordered by dataflow
   deadline; weight blocks land in <=1024-col tiles (a 4096-wide stationary
   tile slows LDWEIGHTS ~2x); y output DMAs are split across queues at the
   tail.
 - global schedule: prologue touching only first-half data (k pair0, q
   pair0/ih0, v blocks 0-7), then 9 attention units with every remaining
   projection and out-projection block injected as PE fillers against
   dataflow deadlines. The last head's second query-half is split into two
   512-wide quarter-units so out-projection blocks 8-11 ride inside the
   final unit; its normalize is split into two 256-col chains feeding
   d12-15. ~75% of HW time is TensorE-busy end to end.
 - per-core partial outputs return bf16 (host sums in f32).

Compute dtype: bf16 matmul inputs, f32 PSUM accumulation, f32->bf16 rope
and softmax intermediates; bf16 cos/sin/rope-bias tables.
"""

import sys
import numpy as np

for _p in ("/opt/trn_rl_repo", "/root/.axon_site/_ro/trn_rl_repo"):
    if _p not in sys.path:
        sys.path.insert(0, _p)

import ml_dtypes

BF16 = ml_dtypes.bfloat16

B, T, D, H = 2, 2048, 1024, 16
HD = 64          # head dim
HG = 4           # heads per core (group)
CQK = 512        # q cols + k cols per core
CV = 256         # v cols per core
N_CORES = 8
KT = 8           # number of 128-row d-chunks (D / 128)


class _Builder:
    """Actual graph construction (kept out of a with-soup for clarity)."""

    def build(self):
        import concourse.mybir as mybir
        from concourse import bacc
        from concourse.tile import TileContext

        f32 = mybir.dt.float32
        bf16 = mybir.dt.bfloat16
        EXP = mybir.ActivationFunctionType.Exp

        nc = bacc.Bacc()

        # wqk/wv come from the host PACKED into 128-partition-row layouts
        # (DMA descriptor rate is ~12M/s per queue, so narrow-row transfers
        # are descriptor-bound; packed rows are 2KB+). wqk column order is
        # [cb2 | cb0 | cb1 | cb3] (deadline order), each block [di, 128].
        xT_e = nc.declare_dram_parameter("xT", [D, T], bf16, isOutput=False)
        wqk_e = nc.declare_dram_parameter("wqk", [128, 4 * 1024], bf16, isOutput=False)
        # rope(b_qk) tables, one [128, T] block per cb in [cb2|cb0|cb1|cb3]
        # order; the q/k bias is added via the rope path (rope is linear)
        ropeb_e = nc.declare_dram_parameter("ropeb", [128, 4 * T], bf16, isOutput=False)
        wv_e = nc.declare_dram_parameter("wv", [128, KT * CV], bf16, isOutput=False)
        bv_e = nc.declare_dram_parameter("bv", [128, CV], bf16, isOutput=False)
        cos_e = nc.declare_dram_parameter("cosT", [128, T], bf16, isOutput=False)
        sin_e = nc.declare_dram_parameter("sinT", [128, T], bf16, isOutput=False)
        ones_e = nc.declare_dram_parameter("ones", [1, T], bf16, isOutput=False)
        wo_e = nc.declare_dram_parameter("wo", [2 * 128, D], bf16, isOutput=False)
        y_e = nc.declare_dram_parameter("y", [T, D], bf16, isOutput=True)

        tc = TileContext(nc)
        tc.__enter__()

        # ---------------- pools ----------------
        pool_live = tc.alloc_tile_pool(name="live", bufs=1)
        pool_a = tc.alloc_tile_pool(name="stageA", bufs=1)
        pool_ps = tc.alloc_tile_pool(name="psum", bufs=1, space="PSUM")

        CH = 1024     # a_chunk column width (one hb half of T)

        # ---------------- input loads ----------------
        # DMA trigger instructions cost ~610ns each on a sequencer; alternate
        # between the two HWDGE issue queues (sync + scalar) so the startup
        # burst is not serialized on one engine.
        _ld_i = [0]

        def load(out, in_):
            eng = nc.sync if (_ld_i[0] % 2 == 0) else nc.scalar
            _ld_i[0] += 1
            eng.dma_start(out=out, in_=in_)

        # Order matches the prologue: a(2,0) [wqk k-pair0 slice + xT hb0]
        # first, then a(0,0) [wqk q-pair0 slice], rope tables hb0, wv for
        # b0-7, then xT hb1 / tables hb1 for a(2,1), late wqk columns for the
        # filler chunks, wo last.
        ones_t = pool_a.tile([1, T], bf16, name="ones_t", tag="ones_t")
        load(ones_t[:, :], ones_e[:, :])

        # warm the ACT exp table set while DMAs stream
        warm_t = pool_a.tile([1, 16], f32, name="warm_t", tag="warm_t")
        nc.scalar.activation(warm_t[:, :], ones_t[0:1, 0:16], EXP)

        # Per-queue DMA BW is only ~24GB/s (a 256KB transfer holds its queue
        # for ~11us), so every bulk load is chunked to <=128KB to spread
        # across the 16 queues.
        # packed wqk SBUF tile; wslice(cb, di) gives the [128, 128] stationary
        # block for matmul (cb in original q0/q1/k0/k1 terms).
        WORD = {2: 0, 0: 1, 1: 2, 3: 3}   # cb -> packed block position
        # one SBUF tile per cb block: stationary matmul reads need a small
        # row pitch (a single packed 4096-wide tile slows LDWEIGHTS ~2x)
        wqk_b = [pool_a.tile([128, 1024], bf16, name=f"wqkb{b}", tag=f"wqkb{b}")
                 for b in range(4)]

        def wslice(cb, di):
            return wqk_b[WORD[cb]][:, di * 128:(di + 1) * 128]

        # k pair0 block: two partition-half DMAs (first matmul gate)
        for p in range(2):
            psl = slice(p * 64, (p + 1) * 64)
            load(wqk_b[0][psl, :], wqk_e[psl, 0:1024])
        xT_t = []
        for di in range(KT):
            xt = pool_a.tile([128, T], bf16, name=f"xT{di}", tag=f"xT{di}")
            for p in range(2):
                psl = slice(p * 64, (p + 1) * 64)
                load(xt[psl, 0:T // 2], xT_e[di * 128 + p * 64:di * 128 + (p + 1) * 64, 0:T // 2])
            xT_t.append(xt)
        # rope tables hb0 next: the slot2 PSUM rotation ties b-unit progress
        # to the rope multiplies, so cos/sin must not trail wv
        cos_t = pool_a.tile([128, T], bf16, name="cos_t", tag="cos_t")
        sin_t = pool_a.tile([128, T], bf16, name="sin_t", tag="sin_t")
        load(cos_t[:, 0:T // 2], cos_e[:, 0:T // 2])
        load(sin_t[:, 0:T // 2], sin_e[:, 0:T // 2])
        # q pair0 block
        for p in range(2):
            psl = slice(p * 64, (p + 1) * 64)
            load(wqk_b[1][psl, :], wqk_e[psl, 1024:2048])
        bv_t = pool_a.tile([128, CV], bf16, name="bv_t", tag="bv_t")
        load(bv_t[:, :], bv_e[:, :])
        wv_all = pool_a.tile([128, KT * CV], bf16, name="wv_all", tag="wv_all")
        for c in range(2):
            csl = slice(c * 1024, (c + 1) * 1024)
            load(wv_all[:, csl], wv_e[:, csl])
        ropeb_b = [pool_a.tile([128, T], bf16, name=f"ropeb{b}", tag=f"ropeb{b}")
                   for b in range(4)]
        for b in range(2):
            for c in range(2):
                csl = slice(c * 1024, (c + 1) * 1024)
                load(ropeb_b[b][:, csl], ropeb_e[:, b * T + c * 1024:b * T + (c + 1) * 1024])
        for di in range(KT):
            for p in range(2):
                load(xT_t[di][p * 64:(p + 1) * 64, T // 2:T],
                     xT_e[di * 128 + p * 64:di * 128 + (p + 1) * 64, T // 2:T])
        load(cos_t[:, T // 2:T], cos_e[:, T // 2:T])
        load(sin_t[:, T // 2:T], sin_e[:, T // 2:T])
        # q pair1 + k pair1 blocks for the filler a_chunks
        load(wqk_b[2][:, :], wqk_e[:, 2048:3072])
        load(wqk_b[3][:, :], wqk_e[:, 3072:4096])
        for b in range(2, 4):
            load(ropeb_b[b][:, :], ropeb_e[:, b * T:(b + 1) * T])

        wo_t = []
        for kb in range(2):
            wt = pool_live.tile([128, D], bf16, name=f"wo{kb}", tag=f"wo{kb}")
            load(wt[:, :], wo_e[kb * 128:(kb + 1) * 128, :])
            wo_t.append(wt)

        # ---------------- persistent tiles ----------------
        # qk8[(cb, hb)]: cb 0/1 = q pair0/pair1 (two heads stacked on
        # partitions), cb 2/3 = k pair0/pair1; hb = T half.
        qk8 = {}
        for cb in range(4):
            for hb in range(2):
                qr = pool_live.tile([128, CH], bf16, name=f"qk{cb}{hb}",
                                    tag=f"qk{cb}{hb}")
                qk8[(cb, hb)] = qr
        v_sb = []
        for tb in range(T // 128):
            vt = pool_live.tile([128, HG, 65], bf16, name=f"v{tb}", tag=f"v{tb}")
            v_sb.append(vt)
        # attn4[(kb, ih)]: attention output transposed, per head-pair kb and
        # query half ih. Rows 0:64 even head, 64:128 odd head.
        attn4 = {}
        for kb in range(2):
            for ih in range(2):
                at = pool_live.tile([128, CH], bf16, name=f"attn{kb}{ih}",
                                    tag=f"attn{kb}{ih}")
                attn4[(kb, ih)] = at
        # q_pad parity tiles: zero half memset once, live half rewritten per
        # unit. Two tiles per parity so back-to-back same-parity units don't
        # serialize on a WAR hazard.
        qpad = {}
        for hh in range(2):
            for j in range(2):
                qp = pool_live.tile([128, CH], bf16, name=f"qpad{hh}{j}",
                                    tag=f"qpad{hh}{j}")
                nc.vector.memset(qp[64 - hh * 64:128 - hh * 64, :], 0.0)
                qpad[(hh, j)] = qp
        for tb in range(T // 128):
            nc.vector.memset(v_sb[tb][:, :, 64:65], 1.0)

        # --- stage A chunk: q/k projection + RoPE for (cb, hb) ---
        def a_chunk(cb, hb):
            ps_qk = pool_ps.tile([128, CH], f32, name="ps2", tag="slot2", bufs=3)
            for half in range(CH // 512):
                c0 = hb * CH + half * 512
                for di in range(KT):
                    nc.tensor.matmul(
                        ps_qk[:, half * 512:(half + 1) * 512],
                        wslice(cb, di),
                        xT_t[di][:, c0:c0 + 512],
                        start=(di == 0), stop=(di == KT - 1),
                    )
            sl = slice(hb * CH, (hb + 1) * CH)
            # rot[p] = q[p]*cos[p] + q[swap(p)]*sin_signed[p]; sin table is
            # pre-swapped-sign so the product can be partition-swapped after
            # the multiply. bf16 temps: half-size swap DMA, 2x DVE add.
            # qsw is pre-filled with the rotated q/k bias (independent of the
            # projection, so it runs before the muls); the swap DMAs then
            # ACCUMULATE into it (gpsimd software DGE does the add).
            qsw = pool_a.tile([128, CH], bf16, name="qsw", tag="qsw", bufs=2)
            nc.vector.tensor_copy(qsw[:, :], ropeb_b[WORD[cb]][:, sl])
            tmps = pool_a.tile([128, CH], bf16, name="tmps", tag="tmps", bufs=2)
            nc.vector.tensor_mul(tmps[:, :], ps_qk[:, :], sin_t[:, sl])
            tmp1 = pool_a.tile([128, CH], bf16, name="ropet1", tag="ropet1", bufs=2)
            nc.vector.tensor_mul(tmp1[:, :], ps_qk[:, :], cos_t[:, sl])
            ADD_OP = mybir.AluOpType.add
            for blk in range(2):
                b0 = blk * 64
                nc.gpsimd.dma_start(out=qsw[b0:b0 + 32, :], in_=tmps[b0 + 32:b0 + 64, :],
                                    accum_op=ADD_OP)
                nc.gpsimd.dma_start(out=qsw[b0 + 32:b0 + 64, :], in_=tmps[b0:b0 + 32, :],
                                    accum_op=ADD_OP)
            nc.vector.tensor_add(qk8[(cb, hb)][:, :], tmp1[:, :], qsw[:, :])

        # --- stage B unit: v projection for one t-block ---
        def b_unit(tb, act=False):
            ps_v = pool_ps.tile([128, CV], f32, name="psv", tag="slot2", bufs=3)
            for di in range(KT):
                nc.tensor.matmul(
                    ps_v[:, :],
                    xT_t[di][:, tb * 128:(tb + 1) * 128],
                    wv_all[:, di * CV:(di + 1) * CV],
                    start=(di == 0), stop=(di == KT - 1),
                )
            # v-bias folded in here: PV then yields ps_o + bv*den, and the
            # normalize turns that into attn + bv (exactly the reference)
            nc.vector.tensor_add(v_sb[tb][:, :, 0:64],
                                 ps_v.rearrange("p (h d) -> p h d", h=HG),
                                 bv_t.rearrange("p (h d) -> p h d", h=HG))

        # --- stage D unit: out projection for one t-block ---
        # y DMA is split into chunks across queues: a single [128, 1024]
        # bf16 DMA is 256KB on ONE queue (~10us at per-queue BW) and would
        # gate the kernel tail.
        def d_unit(tb, act=False, tail=False):
            ih = tb // 8
            tsl = slice((tb % 8) * 128, (tb % 8) * 128 + 128)
            ps_y = pool_ps.tile([128, D], f32, name="psy", tag="slot2", bufs=3)
            for kb in range(2):
                for nb in range(D // 512):
                    nsl = slice(nb * 512, (nb + 1) * 512)
                    nc.tensor.matmul(
                        ps_y[:, nsl],
                        attn4[(kb, ih)][:, tsl],
                        wo_t[kb][:, nsl],
                        start=(kb == 0), stop=(kb == 1),
                    )
            y_sb = pool_live.tile([128, D], bf16, name="y_sb", tag="y_sb", bufs=3)
            with nc.allow_low_precision("bf16 partial output; host sums in f32"):
                if tail == "end":
                    # both engines are free at the tail: halve the copy
                    # latency gating the final output DMAs
                    nc.vector.tensor_copy(y_sb[:, 0:512], ps_y[:, 0:512])
                    nc.scalar.copy(y_sb[:, 512:1024], ps_y[:, 512:1024])
                elif act:
                    nc.scalar.copy(y_sb[:, :], ps_y[:, :])
                else:
                    nc.vector.tensor_copy(y_sb[:, :], ps_y[:, :])
            if tail == "end":
                for p in range(4):
                    psl = slice(p * 32, (p + 1) * 32)
                    eng = nc.sync if p % 2 == 0 else nc.scalar
                    eng.dma_start(out=y_e[tb * 128 + p * 32:tb * 128 + (p + 1) * 32, :],
                                  in_=y_sb[psl, :])
            elif tail:
                for p in range(2):
                    psl = slice(p * 64, (p + 1) * 64)
                    nc.sync.dma_start(out=y_e[tb * 128 + p * 64:tb * 128 + (p + 1) * 64, :],
                                      in_=y_sb[psl, :])
            else:
                nc.sync.dma_start(out=y_e[tb * 128:(tb + 1) * 128, :], in_=y_sb[:, :])

        # --- attention unit (h, ih): scores/exp stream with PV lagging
        # `lag` j-blocks; `fillers` = (jj, closure) PE work injected to soak
        # the ACT-bound slack ---
        IH = 1024
        LAG = 5
        NJ = T // 128

        # cross-boundary pipelining: the first couple of scores+exp of unit i
        # are emitted inside unit i-1's drain phase (keeps ACT streaming and
        # moves the ps_s-buffer stall into PE drain work).
        _pre = {}

        def _emit_score(i, jj):
            h, ih, c0, cw = U[i]
            kt = (qk8[(2 + h // 2, 0)], qk8[(2 + h // 2, 1)])
            jsl = slice((jj % 8) * 128, (jj % 8) * 128 + 128)
            ps_s = pool_ps.tile([128, cw], f32, name="pss", tag="slot2", bufs=3)
            pt = pool_live.tile([128, cw], bf16, name="pt", tag="pt", bufs=LAG + 4)
            for nb in range(cw // 512):
                nsl = slice(nb * 512, (nb + 1) * 512)
                nc.tensor.matmul(ps_s[:, nsl], kt[jj // 8][:, jsl],
                                 qp_tiles[i][:, c0 + nb * 512:c0 + (nb + 1) * 512],
                                 start=True, stop=True)
            nc.scalar.activation(pt[:, :], ps_s[:, :], EXP, scale=0.125)
            return pt

        def pre_scores(i, njj=2):
            _pre[i] = [_emit_score(i, jj) for jj in range(njj)]

        def attn_unit(uidx, fillers=(), lag=None, norm=True):
            h, ih, c0, cw = U[uidx]
            pair, hh = h // 2, h % 2
            csl = slice(c0, c0 + cw)
            fill = sorted(fillers, key=lambda x: x[0])
            if lag is None:
                lag = LAG
            fi = 0
            ps_o = None
            pts = {}
            pre = _pre.pop(uidx, [])
            for jj, pt in enumerate(pre):
                pts[jj] = pt
            for jj in range(NJ + lag):
                if jj < NJ and jj >= len(pre):
                    pts[jj] = _emit_score(uidx, jj)
                while fi < len(fill) and fill[fi][0] <= jj:
                    fill[fi][1]()
                    fi += 1
                if jj >= lag:
                    jb = jj - lag
                    if ps_o is None:
                        ps_o = pool_ps.tile([65, cw], f32, name="pso",
                                            tag="ps_o", bufs=1)
                    pt = pts.pop(jb)
                    last = (jb == NJ - 1)
                    for nb in range(cw // 512):
                        nsl = slice(nb * 512, (nb + 1) * 512)
                        nc.tensor.matmul(ps_o[:, nsl], v_sb[jb][:, h, :],
                                         pt[:, nsl],
                                         start=(jb == 0), stop=last)
            while fi < len(fill):
                fill[fi][1]()
                fi += 1
            if not norm:
                return ps_o
            # normalize rows 0..63 by the denominator (row 64)
            rt = pool_live.tile([65, cw], f32, name="recip_t", tag="recip_t", bufs=2)
            with nc.allow_low_precision("bf16 softmax normalization"):
                nc.vector.reciprocal_approx_fast(out=rt[:, :], in_=ps_o[:, :])
            stg = pool_live.tile([1, cw], f32, name="rstage", tag="rstage", bufs=2)
            nc.sync.dma_start(out=stg[0:1, :], in_=rt[64:65, :])
            rb = pool_live.tile([64, cw], f32, name="rbcast", tag="rbcast", bufs=2)
            nc.gpsimd.partition_broadcast(rb[:, :], stg[0:1, :])
            if hh == 0:
                with nc.allow_low_precision("bf16 attention output"):
                    nc.vector.tensor_mul(attn4[(pair, ih)][0:64, csl],
                                         ps_o[0:64, :], rb[:, :])
            else:
                atmp = pool_live.tile([64, cw], bf16, name="atmp", tag="atmp", bufs=2)
                with nc.allow_low_precision("bf16 attention output"):
                    nc.vector.tensor_mul(atmp[:, :], ps_o[0:64, :], rb[:, :])
                nc.sync.dma_start(out=attn4[(pair, ih)][64:128, csl], in_=atmp[:, :])

        # ---------------- global schedule ----------------
        # Prologue uses only first-half (hb=0) data so it never waits on the
        # late DMA wave; all second-half-dependent work rides as fillers in
        # units 0-1 where the exp stream hides DMA/rope latency. Dataflow
        # deadlines:
        #   a(2,1) k pair0 hb1  -> unit(0,0) jj=8       (filler u0 @1)
        #   b8-11  v blocks     -> unit(0,0) PV jj>=16  (fillers u0)
        #   a(1,0) q pair1 ih0  -> unit(2,0) start      (filler u1 @0)
        #   b12-15 v blocks     -> unit(0,0) PV jj>=20  (fillers u1)
        #   a(3,0) k pair1 hb0  -> unit(2,0) jj=0       (filler u1 @12)
        #   a(3,1) k pair1 hb1  -> unit(2,0) jj=8       (filler u2 @0)
        #   a(0,1) q pair0 ih1  -> unit(0,1) start      (filler u2 @8)
        #   a(1,1) q pair1 ih1  -> unit(3,1) start      (filler u3)
        #   d0-7   out proj ih0 -> ready after unit(3,0); spread 2 per unit
        #   over u4-u7 so even the last units keep PE filler.
        # Each unit's q_pad copy is hoisted into the PREVIOUS unit (@14) so
        # unit boundaries don't serialize on the DVE normalize chain.
        # ih=1 units run (0,1),(1,1),(3,1),(2,1): the LAST unit is an even
        # head whose normalize writes attn directly (no atmp DMA hop on the
        # final critical chain).
        # unit list: (h, ih, c0, cw); the last head's ih=1 half is split into
        # two 512-wide quarter-units so d8-11 ride inside the final unit.
        U = [(0, 0, 0, IH), (1, 0, 0, IH), (2, 0, 0, IH), (3, 0, 0, IH),
             (0, 1, 0, IH), (1, 1, 0, IH), (3, 1, 0, IH),
             (2, 1, 0, 512), (2, 1, 512, 512)]
        qp_tiles = [qpad[(0, 0)], qpad[(1, 0)], qpad[(0, 1)], qpad[(1, 1)],
                    qpad[(0, 0)], qpad[(1, 0)], qpad[(1, 1)],
                    qpad[(0, 1)], qpad[(0, 0)]]

        def qp_copy(i):
            h, ih, c0, cw = U[i]
            pair, hh = h // 2, h % 2
            hp = hh * 64
            csl = slice(c0, c0 + cw)
            nc.vector.tensor_copy(qp_tiles[i][hp:hp + 64, csl],
                                  qk8[(pair, ih)][hp:hp + 64, csl])

        a_chunk(2, 0)
        a_chunk(0, 0)
        qp_copy(0)
        for tb in range(0, 8):
            b_unit(tb, act=True)

        attn_unit(0,
                  fillers=[(1, lambda: a_chunk(2, 1)),
                           (6, lambda: b_unit(8)), (8, lambda: b_unit(9)),
                           (10, lambda: b_unit(10)), (12, lambda: b_unit(11)),
                           (14, lambda: b_unit(12)), (16, lambda: b_unit(13)),
                           (17, lambda: qp_copy(1)),
                           (18, lambda: b_unit(14)), (19, lambda: b_unit(15))])
        attn_unit(1,
                  fillers=[(0, lambda: a_chunk(1, 0)),
                           (8, lambda: a_chunk(3, 0)),
                           (15, lambda: qp_copy(2))])
        attn_unit(2,
                  fillers=[(0, lambda: a_chunk(3, 1)),
                           (8, lambda: a_chunk(0, 1)),
                           (14, lambda: qp_copy(3))])
        attn_unit(3,
                  fillers=[(0, lambda: a_chunk(1, 1)),
                           (14, lambda: qp_copy(4))])
        pool_a.release()
        attn_unit(4,
                  fillers=[(4, lambda: d_unit(0)), (8, lambda: d_unit(1)),
                           (14, lambda: qp_copy(5))])
        attn_unit(5,
                  fillers=[(2, lambda: d_unit(2)), (6, lambda: d_unit(3)),
                           (14, lambda: qp_copy(6))])
        attn_unit(6,
                  fillers=[(2, lambda: d_unit(4)), (6, lambda: d_unit(5)),
                           (10, lambda: d_unit(6)), (14, lambda: qp_copy(7))])
        attn_unit(7,
                  fillers=[(3, lambda: d_unit(7)), (10, lambda: qp_copy(8))])
        # final quarter-unit: d8-11 (query cols 1024:1536, complete once the
        # previous unit's normalize lands) ride inside it; only d12-15 trail.
        ps_of = attn_unit(8, lag=3, norm=False,
                          fillers=[(3, lambda: d_unit(8, tail="mid")),
                                   (6, lambda: d_unit(9, tail="mid")),
                                   (9, lambda: d_unit(10, tail="mid")),
                                   (12, lambda: d_unit(11, tail="mid"))])
        # split final normalize into two 256-col chains, interleaved so the
        # second chain's reciprocal/broadcast overlap the first chain's
        # d_units; gpsimd stays clear of tail DMA issue so the broadcasts
        # aren't queued behind anything.
        rts, stgs, rbs = [], [], []
        for half in range(2):
            rt = pool_live.tile([65, 256], f32, name="recip_f", tag="recip_t", bufs=2)
            with nc.allow_low_precision("bf16 softmax normalization"):
                nc.vector.reciprocal_approx_fast(
                    out=rt[:, :], in_=ps_of[:, half * 256:(half + 1) * 256])
            stg = pool_live.tile([1, 256], f32, name="rstage_f", tag="rstage", bufs=2)
            nc.sync.dma_start(out=stg[0:1, :], in_=rt[64:65, :])
            rb = pool_live.tile([64, 256], f32, name="rbcast_f", tag="rbcast", bufs=2)
            nc.gpsimd.partition_broadcast(rb[:, :], stg[0:1, :])
            rbs.append(rb)
        for half in range(2):
            asl = slice(512 + half * 256, 512 + (half + 1) * 256)
            with nc.allow_low_precision("bf16 attention output"):
                nc.vector.tensor_mul(attn4[(1, 1)][0:64, asl],
                                     ps_of[0:64, half * 256:(half + 1) * 256],
                                     rbs[half][:, :])
            for tb in (12 + 2 * half, 13 + 2 * half):
                d_unit(tb, act=(tb % 2 == 1), tail="end")

        pool_ps.release()
        pool_live.release()
        tc.__exit__(None, None, None)
        nc.finalize()
        return nc


def make_inputs(x, w_qkv, b_qkv, w_out):
    """Host-side shard prep. Returns in_maps list for the 8 cores."""
    half = HD // 2
    inv = 1.0 / (10000.0 ** (np.arange(half, dtype=np.float32) / half))
    fr = np.arange(T, dtype=np.float32)[:, None] * inv[None, :]   # [T, 32]
    cosT = np.cos(fr).T                                           # [32, T]
    sinT = np.sin(fr).T
    cos128 = np.tile(cosT, (4, 1)).astype(np.float32)             # [128, T]
    sin128 = np.tile(sinT, (4, 1)).astype(np.float32)
    sign = np.where((np.arange(128) % 64) < 32, 1.0, -1.0).astype(np.float32)
    sin128 = sin128 * sign[:, None]
    ones_r = np.ones((1, T), dtype=BF16)

    in_maps = []
    for c in range(N_CORES):
        b, g = c // 4, c % 4
        qcols = slice(g * 256, (g + 1) * 256)
        kcols = slice(D + g * 256, D + (g + 1) * 256)
        vcols = slice(2 * D + g * 256, 2 * D + (g + 1) * 256)

        wqk = np.concatenate([w_qkv[:, qcols], w_qkv[:, kcols]], axis=1)  # [D, 512]
        bqk = np.concatenate([b_qkv[qcols], b_qkv[kcols]])                # [512]
        # pack into [128, 4096]: blocks in deadline order [cb2|cb0|cb1|cb3],
        # each block [128, di*128+c] = wqk[di*128+p, cb*128+c]
        blocks = []
        for cb in (2, 0, 1, 3):
            blk = wqk[:, cb * 128:(cb + 1) * 128]                  # [1024, 128]
            blocks.append(blk.reshape(KT, 128, 128).transpose(1, 0, 2).reshape(128, KT * 128))
        wqk_packed = np.concatenate(blocks, axis=1)                # [128, 4096]
        # rope(b_qk) tables: rope is linear, so the q/k bias is applied as a
        # precomputed rotated-bias add on device (kills the bias matmuls).
        # Matches the kernel's rot[p,t] = v[p]cos128[p,t] + v[sw(p)]sin128[sw(p),t].
        sw = (np.arange(128) + 32) % 64 + (np.arange(128) // 64) * 64
        rb_blocks = []
        for cb in (2, 0, 1, 3):
            bvec = bqk[cb * 128:(cb + 1) * 128].astype(np.float32)
            rb = bvec[:, None] * cos128 + bvec[sw][:, None] * sin128[sw, :]
            rb_blocks.append(rb)
        ropeb_packed = np.concatenate(rb_blocks, axis=1)           # [128, 4*T]

        wv = w_qkv[:, vcols]                                          # [D, 256]
        wv_packed = wv.reshape(KT, 128, CV).transpose(1, 0, 2).reshape(128, KT * CV)
        # bv rides as extra columns of the wv pack (a standalone [128,256]
        # load would be descriptor-bound); broadcast across partitions
        bv_aug = np.tile(b_qkv[2 * D + g * 256: 2 * D + (g + 1) * 256].reshape(1, CV),
                         (128, 1))


        wo = w_out[g * 256:(g + 1) * 256, :]                          # [256, D]

        in_maps.append({
            "xT": np.ascontiguousarray(x[b].T).astype(BF16),
            "wqk": np.ascontiguousarray(wqk_packed).astype(BF16),
            "ropeb": np.ascontiguousarray(ropeb_packed).astype(BF16),
            "wv": np.ascontiguousarray(wv_packed).astype(BF16),
            "bv": bv_aug.astype(BF16),
            "cosT": cos128.astype(BF16),
            "sinT": sin128.astype(BF16),
            "ones": ones_r,
            "wo": np.ascontiguousarray(wo).astype(BF16),
        })
    return in_maps


_NC_CACHE = [None]


def get_graph():
    if _NC_CACHE[0] is None:
        _NC_CACHE[0] = _Builder().build()
    return _NC_CACHE[0]


def kernel(x, w_qkv, b_qkv, w_out, b_out, _trace=False):
    from concourse.bass_utils import run_bass_kernel_spmd

    x = np.asarray(x)
    w_qkv = np.asarray(w_qkv)
    b_qkv = np.asarray(b_qkv)
    w_out = np.asarray(w_out)
    b_out = np.asarray(b_out)

    nc = get_graph()
    in_maps = make_inputs(x, w_qkv, b_qkv, w_out)
    kw = {}
    if _trace:
        _install_ntff_shim()
        kw = {"trace": True}
    res = run_bass_kernel_spmd(nc, in_maps, core_ids=list(range(N_CORES)), **kw)

    out = np.empty((B, T, D), dtype=np.float32)
    for b in range(B):
        acc = np.asarray(res.results[4 * b]["y"]).astype(np.float32)
        for g in range(1, 4):
            acc += np.asarray(res.results[4 * b + g]["y"]).astype(np.float32)
        out[b] = acc + b_out[None, :]
    if _trace:
        kernel.last_exec_time_ns = res.exec_time_ns
        kernel.last_result = res
    return out


def _install_ntff_shim():
    """The agent image's antenv lacks axon_hooks; shim it so trace=True works."""
    import types
    if "antenv.axon_hooks" in sys.modules:
        return
    try:
        from trn_agent_boot.trn_boot import _ntff_profile_via_ctypes
        hook = _ntff_profile_via_ctypes("/opt/axon/libaxon_pjrt.so")
    except Exception:
        hook = None
    mod = types.ModuleType("antenv.axon_hooks")
    _h = [hook]
    mod.set_axon_ntff_profile_hook = lambda h: _h.__setitem__(0, h)
    mod.get_axon_ntff_profile_hook = lambda: _h[0]
    sys.modules["antenv.axon_hooks"] = mod
